# revision 101
# baseline (speedup 1.0000x reference)
"""Self-contained Trainium2 Bass kernel for MultiHeadAttention.

Problem: B=2, S=2048, D=1024, H=16, hd=64, with the reference's
masked_fill(mask==0, -1e-09) quirk: masked scores become ~0.0, so
exp(masked) == 1.0 in fp32 and every key position participates in the
softmax denominator. Fully-masked key blocks therefore contribute a
block-constant suffix sum of V rows, added via cheap rank-1-style
matmuls instead of full score/attn matmuls.

Sharding: 8 cores = 2 batches x 4 head-groups (4 heads per core).
Each core computes a partial [S, D] output (its 4 heads pushed through
the O-projection); the host sums the 4 partials per batch and adds bo.

Layouts (per core, all matmul operands at partition base 0):
  qt  [128, pair, S]   q^T, two heads stacked on partitions (d dims)
  ktz [128, head, S]   k^T zero-padded: even heads live on partitions
                       0-63 (64-127 zero), odd heads on 64-127 - the
                       scores matmul is then a plain K=128 matmul
                       against the pair-stacked qt.
  v2  [128, head, kj, 65]  V blocks with an appended ones column
                       (produces the softmax denominator for free).
  scores^T [sk, sq] in PSUM -> exp on ScalarE -> bf16 tiles ->
  attnU^T [65, sq] accumulated with V2 stationary (N=512 moving);
  rowsum = row 64.  1/rowsum = exp(-ln(rowsum)) on ScalarE (one shared
  Exp+Ln table set, patched below), written in bf16 into a pre-zeroed
  SBUF tile; a plain K=128 ones-stationary matmul then broadcasts it
  across partitions (no HBM round-trip, no gpsimd ucode).

Scheduling: the PE instruction queue is FIFO in emission order, so the
kj loop is software-pipelined depth-2 (scores(i) is emitted before
attnU(i-2), hiding the ScalarE exp + gpsimd affine_select latency) and
independent projection / output-projection / finalize units are
interleaved as fillers so the PE never waits on the exp chain. Input
DMAs are merged into few large transfers, staged in waves via WAR
dependencies (one-column overlap with the previous wave's readers) so
the SDMA packet round-robin cannot starve the first wave. Dummy
matmuls keep the PE's HAM activity window hot through the DMA-bound
prologue and the finalize tail.
"""

import numpy as np
import ml_dtypes

import concourse.bass as bass
import concourse.bacc as bacc
import concourse.tile as tile
import concourse.mybir as mybir
from concourse import library_config
from concourse.bass_utils import run_bass_kernel_spmd

BF16 = mybir.dt.bfloat16
F32 = mybir.dt.float32
NPBF16 = ml_dtypes.bfloat16
AF = mybir.ActivationFunctionType

B = 2
S = 2048
D = 1024
H = 16
HD = 64
NCORES = 8
HPC = 4            # heads per core
NPAIRS = 2         # head pairs per core
NQ = S // 128      # 16 query/key blocks of 128
QCH = 512          # sq chunk width
NCH = S // QCH     # 4 chunks
KT = D // 128      # 8 contraction tiles for projections
import os
NWARM = int(os.environ.get("NWARM", "72"))  # PE warmup matmuls
NTAILWARM = int(os.environ.get("NTAILWARM", "22"))
BCAST = os.environ.get("BCAST", "pemm")  # pemm | gpsimd | dma
ROWTILE = os.environ.get("ROWTILE", "0") == "1"
RECIP = os.environ.get("RECIP", "act")  # act | div | dve | dvenat


def _emit(tc: tile.TileContext, io: dict):
    nc = tc.nc

    persist = tc.alloc_tile_pool(name="persist", bufs=1)

    # ---- constants ----
    ones128 = persist.tile([128, 128], BF16, name="ones128")
    nc.gpsimd.memset(ones128, 1.0)
    ones64b = persist.tile([128, 64], BF16, name="ones64b")
    nc.gpsimd.memset(ones64b, 1.0)
    onesrow = persist.tile([128, 2, QCH], BF16, name="onesrow")
    nc.gpsimd.memset(onesrow[64:65, :, :], 1.0)

    # Prefetch the Exp activation table (~2.7us) while input DMAs stream.
    actwarm = persist.tile([1, 8], F32, name="actwarm")
    nc.vector.memset(actwarm, 0.0)
    actwarm2 = persist.tile([1, 8], F32, name="actwarm2")
    nc.scalar.activation(actwarm2[0:1, :], actwarm[0:1, :], AF.Exp)

    if BCAST == "gpsimd":
        # gpsimd library with partition_broadcast (used by finalize)
        nc.gpsimd.load_library(library_config.attn)

    # ---- persistent SBUF arrays ----
    qt = persist.tile([128, NPAIRS, S], BF16, name="qt")
    if ROWTILE:
        kt = persist.tile([128, NPAIRS, S], BF16, name="kt")
    else:
        ktz = persist.tile([128, HPC, S], BF16, name="ktz")
        for h in range(HPC):  # zero the unused half of each ktz head
            half = slice(64, 128) if h % 2 == 0 else slice(0, 64)
            nc.gpsimd.memset(ktz[half, h, :], 0.0)
    v2 = persist.tile([128, HPC, NQ, 65], BF16, name="v2")
    fs = persist.tile([128, HPC, NQ, 65], BF16, name="fs")
    att = persist.tile([128, NPAIRS, S], BF16, name="att")

    qts = persist.tile([128, KT, S], BF16, name="qts")
    kts = persist.tile([128, KT, S], BF16, name="kts")
    vts = persist.tile([128, KT, S], BF16, name="vts")
    wqt = persist.tile([128, KT, 256], BF16, name="wqt")
    wkt = persist.tile([128, KT, 256], BF16, name="wkt")
    wvt = persist.tile([128, KT, 256], BF16, name="wvt")
    wot = persist.tile([128, NPAIRS, D], BF16, name="wot")
    # q/k biases as per-partition columns (dims live on partitions in
    # the qt/ktz layout): bqp[:, p] is the [128, 1] bias for pair p,
    # added during the PSUM->SBUF cast via tensor_scalar - no rank-1
    # bias matmuls on the PE.
    bqp = persist.tile([128, NPAIRS], F32, name="bqp")
    bkp = persist.tile([128, NPAIRS], F32, name="bkp")

    # ---- input DMAs: few, large transfers; spread across engine queues
    # so no compute engine's FIFO gets blocked behind a slow trigger.
    # V-side first (vproj runs first), then QK wave 1, then wave 2.
    vt3 = io["VT"].rearrange("(t p) s -> p t s", t=KT)
    qt3 = io["QT"].rearrange("(t p) s -> p t s", t=KT)
    kt3 = io["KT"].rearrange("(t p) s -> p t s", t=KT)
    # vts wave 1 is the critical first data (vproj runs first): split it
    # across the sync and scalar HWDGE rings so it gets ~2x the
    # per-ring bandwidth share during the congested prologue.
    nc.sync.dma_start(wvt[:, :, :], io["WvT"].rearrange("(t p) n -> p t n", t=KT))
    nc.sync.dma_start(vts[:, 0:4, 0:QCH], vt3[:, 0:4, 0:QCH])
    nc.scalar.dma_start(vts[:, 4:8, 0:QCH], vt3[:, 4:8, 0:QCH])
    nc.gpsimd.dma_start(wqt[:, :, :], io["WqT"].rearrange("(t p) n -> p t n", t=KT))
    nc.gpsimd.dma_start(wkt[:, :, :], io["WkT"].rearrange("(t p) n -> p t n", t=KT))
    nc.scalar.dma_start(bqp, io["bqT"].rearrange("g p -> p g"))
    nc.scalar.dma_start(bkp, io["bkT"].rearrange("g p -> p g"))
    nc.scalar.dma_start(qts[:, :, 0:QCH], qt3[:, :, 0:QCH])
    nc.gpsimd.dma_start(kts[:, :, 0:QCH], kt3[:, :, 0:QCH])
    nc.scalar.dma_start(wot[:, :, :],
                        io["WoT"].rearrange("(g p) n -> p g n", g=NPAIRS))
    # Later waves are staged so they can't steal HBM bandwidth from the
    # critical wave-1 data (the SDMA engines round-robin over ALL queued
    # DMAs at packet granularity). Each wave's dst overlaps its
    # predecessor by ONE column: tiny Vector "dummy reads" of that
    # column complete the moment wave 1 lands, and the WAR dependency on
    # them releases the qk wave-2 triggers right then (~20us); the final
    # qk wave chains behind wave 2 via the same-column WAW. vts wave 2
    # stays anchored on vproj(0)'s reads.
    def dma_v_wave2():
        nc.sync.dma_start(vts[:, :, QCH - 1:], vt3[:, :, QCH - 1:])

    def dma_qk_wave(w):
        s = w * QCH - 1
        e = (w + 1) * QCH
        nc.sync.dma_start(qts[:, :, s:e], qt3[:, :, s:e])
        nc.gpsimd.dma_start(kts[:, :, s:e], kt3[:, :, s:e])

    nc.gpsimd.memset(v2[:, :, :, 64:65], 1.0)  # ones column
    nc.gpsimd.memset(fs[:, :, NQ - 1, :], 0.0)  # suffix chain seed

    pb_s = tc.alloc_tile_pool(name="pb_scores", bufs=2, space="PSUM")
    pb_a = tc.alloc_tile_pool(name="pb_attnu", bufs=2, space="PSUM")

    # PE warmup: the first ~7us are DMA-bound with the PE idle, so the
    # HAM clock gate holds the array at 1.2 GHz when real work starts.
    # Dummy matmuls on the resident ones128 keep the activity window hot
    # (outputs are never read).
    for i in range(NWARM):
        w = pb_s.tile([128, 2, QCH], F32, tag="sps", name=f"warm{i}")
        nc.tensor.matmul(w[:, 0, 0:128], ones128, ones128,
                         start=True, stop=True)
    pb_e = tc.alloc_tile_pool(name="pb_exp", bufs=7)
    pb_r = tc.alloc_tile_pool(name="pb_recip", bufs=2)
    rec_bufs = []
    if BCAST == "pemm":
        # rec buffers: only row 64 is ever written (the reciprocal); the
        # other 127 rows stay zero so a plain K=128 ones-stationary
        # matmul broadcasts row 64 across output partitions. Two
        # persistent buffers, alternated across finalize_pair calls.
        for i in range(2):
            z = persist.tile([128, 2, QCH], BF16, name=f"recbuf{i}")
            nc.gpsimd.memset(z, 0.0)
            rec_bufs.append(z)
    rec_idx = [0]

    def vproj_unit(st):
        """V projection for key block st -> v2 tiles (one strided cast)."""
        psv_t = pb_s.tile([128, 2, 4, 64], F32, tag="sps", name=f"ps_v{st}")
        ps_v = psv_t[:, 0, :, :]
        for t in range(KT):
            nc.tensor.matmul(ps_v,
                             vts[:, t, st * 128:(st + 1) * 128],
                             wvt[:, t, :], start=(t == 0),
                             stop=(t == KT - 1))
        nc.vector.tensor_copy(v2[:, :, st, 0:64], ps_v)

    def vproj(c):
        for st in range(4 * c, 4 * c + 4):
            vproj_unit(st)

    def qproj_unit(c, p):
        sq = slice(c * QCH, (c + 1) * QCH)
        psq_t = pb_s.tile([128, 2, QCH], F32, tag="sps", name=f"ps_q{p}_{c}")
        ps_q = psq_t[:, 0, :]
        for t in range(KT):
            nc.tensor.matmul(ps_q, wqt[:, t, p * 128:(p + 1) * 128],
                             qts[:, t, sq], start=(t == 0),
                             stop=(t == KT - 1))
        nc.vector.tensor_scalar_add(qt[:, p, sq], ps_q, bqp[:, p:p + 1])

    def kproj_unit(c, p):
        sq = slice(c * QCH, (c + 1) * QCH)
        psk_t = pb_s.tile([128, 2, QCH], F32, tag="sps", name=f"ps_k{p}_{c}")
        ps_k = psk_t[:, 0, :]
        for t in range(KT):
            nc.tensor.matmul(ps_k, wkt[:, t, p * 128:(p + 1) * 128],
                             kts[:, t, sq], start=(t == 0),
                             stop=(t == KT - 1))
        if ROWTILE:
            nc.vector.tensor_scalar_add(kt[:, p, sq], ps_k, bkp[:, p:p + 1])
        else:
            nc.vector.tensor_scalar_add(ktz[0:64, 2 * p, sq], ps_k[0:64, :],
                                        bkp[0:64, p:p + 1])
            nc.vector.tensor_scalar_add(ktz[64:128, 2 * p + 1, sq],
                                        ps_k[64:128, :],
                                        bkp[64:128, p:p + 1])

    def qkproj(c):
        for p in range(NPAIRS):
            qproj_unit(c, p)
            kproj_unit(c, p)

    def qk_fillers(c):
        return [lambda p=p, f=f: f(c, p)
                for p in range(NPAIRS) for f in (qproj_unit, kproj_unit)]

    def fs_segment(qlo, qhi, split=False):
        """fs[q] for q = qhi-1 .. qlo (needs v2[qlo+1..qhi] and fs[qhi]).
        vproj runs in reverse block order (3,2,1) so these segments
        build incrementally, each right after the v2 blocks it needs.
        The last (longest) segment splits heads across Vector/GpSimd."""
        for h in range(HPC):
            eng = nc.gpsimd if (split and h >= 2) else nc.vector
            for q in range(qhi - 1, qlo - 1, -1):
                eng.tensor_add(fs[:, h, q, :], fs[:, h, q + 1, :],
                               v2[:, h, q + 1, :])

    aups_tiles = {}

    def chunk_loop(c, fillers=(), pre_special=None, tailwarm=0,
                   early_fillers=()):
        """scores -> exp -> attnU^T for chunk c, both pairs.

        Software-pipelined: scores+exp for unit i+1 are emitted before
        attnU of unit i, so the PE streams the next scores while the
        ScalarE exp chain drains.  Filler units (independent PE work)
        are spread evenly through the loop to absorb the exp backlog.

        finalize work is emitted per-pair: pair 0's finalize lands a few
        units into pair 1's run (hiding its Ln/Exp latency behind live
        PE work); pair 1's finalize is handed to the NEXT chunk loop via
        pre_special (or emitted at the end for the last chunk).
        """
        fillers = list(fillers)
        nkj = 4 * c + 4
        units = [(p, kj) for p in range(NPAIRS) for kj in range(nkj)]
        nu = len(units)
        fill_at = {}
        for j, f in enumerate(early_fillers):  # fixed positions 0,1,2..
            fill_at.setdefault(min(nu - 1, j), []).append(f)
        if pre_special is not None:  # e.g. previous chunk's p1 finalize
            fill_at.setdefault(min(nu - 1, 2), []).append(pre_special)
        if c > 0:  # this chunk's p0 finalize, 3 units into p1's run
            fill_at.setdefault(min(nu - 1, nkj + 2), []).append(
                lambda: finalize_pair(c, 0))
        # regular fillers spread from position 3 onward
        if fillers:
            for j, f in enumerate(fillers):
                fill_at.setdefault(
                    min(nu - 1, 3 + (j * (nu - 3)) // len(fillers)), []).append(f)

        exts = {}

        def emit_scores_exp(p, kj):
            c0 = max(kj - 4 * c, 0) * 128   # first valid col in chunk
            sps = pb_s.tile([128, 2, QCH], F32, tag="sps",
                            name=f"sps{p}_{c}_{kj}")
            for hl in range(2):
                h0 = hl * 64
                if ROWTILE:
                    # K=64 row-tiled: the two heads run concurrently in
                    # the PE array (rows 0-63 / 64-127).
                    nc.tensor.matmul(
                        sps[:, hl, c0:QCH],
                        kt[h0:h0 + 64, p, kj * 128:(kj + 1) * 128],
                        qt[h0:h0 + 64, p, c * QCH + c0:(c + 1) * QCH],
                        start=True, stop=True)
                else:
                    nc.tensor.matmul(
                        sps[:, hl, c0:QCH],
                        ktz[:, 2 * p + hl, kj * 128:(kj + 1) * 128],
                        qt[:, p, c * QCH + c0:(c + 1) * QCH],
                        start=True, stop=True)
            ext = pb_e.tile([128, 2, QCH], BF16, tag="ext",
                            name=f"ext{p}_{c}_{kj}")
            nc.scalar.activation(ext[:, :, c0:QCH], sps[:, :, c0:QCH],
                                 AF.Exp, scale=0.125)
            if kj >= 4 * c:  # diagonal block: masked exp entries -> 1.0
                # one affine_select covers both heads: the hl dim gets
                # a zero stride in the affine pattern
                nc.gpsimd.affine_select(
                    out=ext[:, :, c0:c0 + 128],
                    in_=ext[:, :, c0:c0 + 128],
                    compare_op=mybir.AluOpType.is_ge,
                    fill=1.0, base=0,
                    pattern=[[0, 2], [1, 128]], channel_multiplier=-1)
            exts[(p, kj)] = (ext, c0)

        def emit_attnu(p, kj):
            ext, c0 = exts.pop((p, kj))
            if kj == 0:
                # allocate here (not at scores emission) so a chunk's
                # first scores/exp aren't gated on the previous chunk's
                # finalize muls releasing this pool slot
                aups_tiles[(p, c)] = pb_a.tile(
                    [128, 2, QCH], F32, tag="aups", name=f"aups{p}_{c}")
            aups = aups_tiles[(p, c)]
            for hl in range(2):
                # masked cols < c0 get their (block-constant)
                # contribution from the early FS matmuls below
                nc.tensor.matmul(
                    aups[0:65, hl, c0:QCH],
                    v2[:, 2 * p + hl, kj, :],
                    ext[:, hl, c0:QCH],
                    start=(kj == 0),
                    stop=(kj == nkj - 1 and c > 0))
            if kj == 0 and c > 0:
                # suffix adds commute with the accumulation: emit them
                # up front so finalize()'s recip can start the moment
                # the last attnU matmul lands
                for hl in range(2):
                    h = 2 * p + hl
                    for ql in range(4):
                        qi = 4 * c + ql
                        if qi < NQ - 1:
                            nc.tensor.matmul(
                                aups[0:65, hl, ql * 128:(ql + 1) * 128],
                                fs[:, h, qi, :], ones128,
                                start=False, stop=False)

        # depth-2 software pipeline: attnU(i-2) is emitted after
        # scores(i), covering the ScalarE exp AND the gpsimd
        # affine_select latency of the diagonal blocks.
        for i, (p, kj) in enumerate(units):
            emit_scores_exp(p, kj)
            if i > 1:
                emit_attnu(*units[i - 2])
            for f in fill_at.get(i, ()):
                f()
        emit_attnu(*units[-2])
        emit_attnu(*units[-1])
        for i in range(tailwarm):
            w = pb_s.tile([128, 2, QCH], F32, tag="sps", name=f"twarm{c}_{i}")
            nc.tensor.matmul(w[:, 0, 0:128], ones128, ones128,
                             start=True, stop=True)

    def fs_close(c, p):
        """fs suffix matmuls closing the aups accumulation for chunk 0
        (fs is not yet computed when chunk 0's attnU is emitted)."""
        aups = aups_tiles[(p, c)]
        for hl in range(2):
            for ql in range(4):
                nc.tensor.matmul(
                    aups[0:65, hl, ql * 128:(ql + 1) * 128],
                    fs[:, 2 * p + hl, 4 * c + ql, :], ones128,
                    start=False, stop=(ql == 3))

    def finalize_pair(c, p, blockwise=False, fs_done=False):
        """rowsum reciprocal (ScalarE Ln/Exp) -> partition broadcast via
        a K=1 PE matmul into the UNUSED partitions 64-127 of the aups
        PSUM tile -> normalize into att for chunk c, pair p."""
        ch = slice(c * QCH, (c + 1) * QCH)
        aups = aups_tiles[(p, c)]
        if c == 0 and not fs_done:
            for hl in range(2):
                for ql in range(4):
                    nc.tensor.matmul(
                        aups[0:65, hl, ql * 128:(ql + 1) * 128],
                        fs[:, 2 * p + hl, 4 * c + ql, :], ones128,
                        start=False, stop=(ql == 3))
        if BCAST == "pemm":
            rec = rec_bufs[rec_idx[0] % 2]
            rec_idx[0] += 1
        else:
            rec = pb_r.tile([128, 2, QCH], F32, tag="lnr", name=f"rec{p}_{c}")
        if RECIP == "div":
            # single DVE divide: keeps the reciprocal chain off the
            # ScalarE queue, which is saturated with softmax exps
            nc.vector.tensor_tensor(rec[64:65, :, :], onesrow[64:65, :, :],
                                    aups[64:65, :, :],
                                    mybir.AluOpType.divide)
        else:
            lnr = pb_r.tile([128, 2, QCH], F32, tag="lnr", name=f"lnr{p}_{c}")
            nc.scalar.activation(lnr[64:65, :, :], aups[64:65, :, :], AF.Ln)
            nc.scalar.activation(rec[64:65, :, :], lnr[64:65, :, :],
                                 AF.Exp, scale=-1.0)
        if BCAST == "pemm":
            # broadcast 1/rowsum to all partitions: rows != 64 of rec
            # are zero, so ones128^T @ rec replicates row 64. Standard
            # (0,0) matmul into a borrowed scores-pool PSUM tile, then a
            # DVE copy to SBUF for the (PSUM x SBUF) normalize muls.
            rep_ps = pb_s.tile([128, 2, QCH], F32, tag="sps",
                               name=f"repps{p}_{c}")
            for hl in range(2):
                nc.tensor.matmul(rep_ps[:, hl, :], ones128, rec[:, hl, :],
                                 start=True, stop=True)
            rep = pb_r.tile([128, 2, QCH], BF16, tag="rep", name=f"rep{p}_{c}")
            nc.vector.tensor_copy(rep[0:64, :, :], rep_ps[0:64, :, :])
            rl = 0
        else:
            rep = pb_r.tile([128, 2, QCH], F32, tag="rep", name=f"rep{p}_{c}")
            r = p * NCH + c
            nc.sync.dma_start(io["dscratch"][r:r + 1, :], rec[64:65, :, :])
            nc.sync.dma_start(
                rep[0:64, :, :],
                io["dscratch"][r:r + 1, :].rearrange(
                    "r (h q) -> r h q", h=2).broadcast_to([64, 2, QCH]))
            rl = 0
        if blockwise:
            # per-128-query-block muls so the tail outproj units can
            # start as soon as their block is normalized
            for ql in range(4):
                cqs = slice(c * QCH + ql * 128, c * QCH + (ql + 1) * 128)
                for hl in range(2):
                    nc.vector.tensor_mul(
                        att[hl * 64:(hl + 1) * 64, p, cqs],
                        aups[0:64, hl, ql * 128:(ql + 1) * 128],
                        rep[rl:rl + 64, hl, ql * 128:(ql + 1) * 128])
        else:
            for hl in range(2):
                nc.vector.tensor_mul(
                    att[hl * 64:(hl + 1) * 64, p, ch],
                    aups[0:64, hl, :],
                    rep[rl:rl + 64, hl, :])

    def finalize(c, blockwise=False):
        for p in range(NPAIRS):
            finalize_pair(c, p, blockwise)

    ob_tiles = {}

    def outproj_unit(st, dc):
        pso = pb_s.tile([128, 2, QCH], F32, tag="sps", name=f"pso{st}_{dc}")
        for p in range(NPAIRS):
            # K=128 contraction = both heads of the pair stacked
            nc.tensor.matmul(
                pso[:, 0, :],
                att[:, p, st * 128:(st + 1) * 128],
                wot[:, p, dc * 512:(dc + 1) * 512],
                start=(p == 0), stop=(p == NPAIRS - 1))
        if st not in ob_tiles:
            ob_tiles[st] = pb_e.tile([128, 2, QCH], BF16, tag="ob",
                                     name=f"ob{st}")
        ob = ob_tiles.pop(st) if dc == 1 else ob_tiles[st]
        if st >= 12:  # tail blocks: ScalarE is idle after the last exp,
            # so do the PSUM->SBUF casts there and keep VectorE free for
            # the final normalize muls
            nc.scalar.copy(ob[:, dc, :], pso[:, 0, :])
        else:
            nc.vector.tensor_copy(ob[:, dc, :], pso[:, 0, :])
        if dc == 1:  # one output DMA per 128-row block
            nc.gpsimd.dma_start(io["out"][st * 128:(st + 1) * 128, :], ob)

    def outproj(c):
        for st in range(4 * c, 4 * c + 4):
            for dc in range(2):
                outproj_unit(st, dc)

    def op_fillers(c):
        return [lambda st=st, dc=dc: outproj_unit(st, dc)
                for st in range(4 * c, 4 * c + 4) for dc in range(2)]

    # Main schedule.  finalize(c) needs ALL v2 blocks (its FS suffix
    # sums reach to kj=15 - a true data dependency of the reference), so
    # finalizes wait until all v-projections are emitted; aups
    # double-buffering then bounds how many chunk loops may run before
    # the first finalize.  qkproj(c+1) and outproj(c-1) units are
    # interleaved into chunk_loop(c) as PE fillers.
    vproj(0)
    dma_v_wave2()
    qkproj(0)
    dma_qk_wave(1)
    chunk_loop(0)
    vproj(3)
    fs_segment(11, NQ - 1)
    vproj(2)
    fs_segment(7, 11)
    vproj(1)
    fs_segment(0, 7, split=True)
    # only qproj(1, p0) is needed before chunk 1's first scores (keys
    # kj 0..3 come from kproj(0)); the other three qkproj(1) units run
    # as fixed-position early fillers INSIDE chunk 1, so its exp stream
    # starts ~5us sooner and the boundary loses its ScalarE wait.
    qproj_unit(1, 0)
    # emit BOTH pairs' fs-closing matmuls before any recip chain, so
    # pair 1's PE work fills the Ln/Exp latency of pair 0's reciprocal
    fs_close(0, 0)
    fs_close(0, 1)
    for i in range(10):  # keep the PE's HAM window hot through the
        # first reciprocal chain (bc matmuls wait on ScalarE here)
        w = pb_s.tile([128, 2, QCH], F32, tag="sps", name=f"f0warm{i}")
        nc.tensor.matmul(w[:, 0, 0:128], ones128, ones128,
                         start=True, stop=True)
    finalize_pair(0, 0, fs_done=True)
    finalize_pair(0, 1, fs_done=True)
    # filler balance: chunks 1/2 are close to PE-bound already (each
    # exp-paced unit leaves only ~150ns of PE slack), while chunk 3 has
    # 32 exp-paced units - park the output-projection work there.
    chunk_loop(1, early_fillers=[lambda: kproj_unit(1, 0),
                                 lambda: qproj_unit(1, 1),
                                 lambda: kproj_unit(1, 1),
                                 lambda: dma_qk_wave(2)],
               fillers=qk_fillers(2) + [lambda: dma_qk_wave(3)]
               + op_fillers(0))
    chunk_loop(2, fillers=qk_fillers(3),
               pre_special=lambda: finalize_pair(1, 1))
    chunk_loop(3, fillers=op_fillers(1) + op_fillers(2),
               pre_special=lambda: finalize_pair(2, 1), tailwarm=NTAILWARM)
    finalize_pair(3, 1, blockwise=True)
    outproj(3)

    pb_r.release()
    pb_e.release()
    pb_a.release()
    pb_s.release()
    persist.release()


_CACHED = None


def _patch_act_tables():
    """Make Exp and Ln resolve to the single combined table set so the
    per-chunk recip (Ln/Exp) doesn't thrash ACT_TABLE_LOADs against the
    softmax Exp calls. Set positions (= act_func_set_id) are preserved;
    only membership of Exp/Ln in other sets is hidden from the selector."""
    from concourse import hw_specs
    orig = hw_specs.get_activation_tables

    def patched(arch):
        t = dict(orig(arch))
        if "natural_log_exp_and_others" in t:
            for name in t:
                if name != "natural_log_exp_and_others":
                    t[name] = t[name] - {AF.Exp, AF.Ln}
        return t

    bacc.get_activation_tables = patched


def _build():
    global _CACHED
    if _CACHED is not None:
        return _CACHED
    _patch_act_tables()
    nc = bacc.Bacc("TRN2", target_bir_lowering=False, debug=False)
    io = {
        "QT": nc.dram_tensor("QT", [D, S], BF16, kind="ExternalInput").ap(),
        "KT": nc.dram_tensor("KT", [D, S], BF16, kind="ExternalInput").ap(),
        "VT": nc.dram_tensor("VT", [D, S], BF16, kind="ExternalInput").ap(),
        "WqT": nc.dram_tensor("WqT", [D, 256], BF16, kind="ExternalInput").ap(),
        "WkT": nc.dram_tensor("WkT", [D, 256], BF16, kind="ExternalInput").ap(),
        "WvT": nc.dram_tensor("WvT", [D, 256], BF16, kind="ExternalInput").ap(),
        "WoT": nc.dram_tensor("WoT", [256, D], BF16, kind="ExternalInput").ap(),
        "bqT": nc.dram_tensor("bqT", [NPAIRS, 128], F32,
                              kind="ExternalInput").ap(),
        "bkT": nc.dram_tensor("bkT", [NPAIRS, 128], F32,
                              kind="ExternalInput").ap(),
        "out": nc.dram_tensor("out", [S, D], BF16, kind="ExternalOutput").ap(),
        "dscratch": nc.dram_tensor("dscratch", [NPAIRS * NCH, 2 * QCH], F32,
                                   kind="Internal").ap(),
    }
    with tile.TileContext(nc) as tc:
        _emit(tc, io)
    nc.compile()
    _CACHED = (nc, io)
    return _CACHED


def make_in_maps(Q, K, V, Wq, bq, Wk, bk, Wv, bv, Wo):
    """Build the 8 per-core input dicts (host-side sharding)."""
    Q = np.asarray(Q, np.float32)
    K = np.asarray(K, np.float32)
    V = np.asarray(V, np.float32)
    qt = [np.ascontiguousarray(Q[b].T).astype(NPBF16) for b in range(B)]
    kt = [np.ascontiguousarray(K[b].T).astype(NPBF16) for b in range(B)]
    vt = [np.ascontiguousarray(V[b].T).astype(NPBF16) for b in range(B)]
    in_maps = []
    for core in range(NCORES):
        b, g = divmod(core, 4)
        rows = slice(g * 256, (g + 1) * 256)
        in_maps.append({
            "QT": qt[b], "KT": kt[b], "VT": vt[b],
            "WqT": np.ascontiguousarray(np.asarray(Wq, np.float32)[rows].T).astype(NPBF16),
            "WkT": np.ascontiguousarray(np.asarray(Wk, np.float32)[rows].T).astype(NPBF16),
            "WvT": np.ascontiguousarray(np.asarray(Wv, np.float32)[rows].T).astype(NPBF16),
            "WoT": np.ascontiguousarray(np.asarray(Wo, np.float32)[:, rows].T).astype(NPBF16),
            "bqT": np.ascontiguousarray(
                np.asarray(bq, np.float32)[rows].reshape(NPAIRS, 128)),
            "bkT": np.ascontiguousarray(
                np.asarray(bk, np.float32)[rows].reshape(NPAIRS, 128)),
        })
    return in_maps


def kernel(Q, K, V, mask, Wq, bq, Wk, bk, Wv, bv, Wo, bo, _results_hook=None):
    nc, _io = _build()
    in_maps = make_in_maps(Q, K, V, Wq, bq, Wk, bk, Wv, bv, Wo)
    res = run_bass_kernel_spmd(nc, in_maps, core_ids=list(range(NCORES)))
    if _results_hook is not None:
        _results_hook(res)
    out = np.zeros((B, S, D), np.float32)
    for core in range(NCORES):
        out[core // 4] += np.asarray(res.results[core]["out"], np.float32)
    # bv passes straight through the softmax average; fold it (and bo)
    # into the output bias here.
    out += np.asarray(bo, np.float32) + \
        np.asarray(bv, np.float32) @ np.asarray(Wo, np.float32).T
    return out


# revision 103
# speedup vs baseline: 1.2259x; 1.2259x over previous
"""Self-contained Trainium2 Bass kernel for MultiHeadAttention.

Problem: B=2, S=2048, D=1024, H=16, hd=64, with the reference's
masked_fill(mask==0, -1e-09) quirk: masked scores become ~0.0, so
exp(masked) == 1.0 in fp32 and every key position participates in the
softmax denominator. Fully-masked key blocks therefore contribute a
block-constant suffix sum of V rows, added via cheap rank-1-style
matmuls instead of full score/attn matmuls.

Sharding: 8 cores = 2 batches x 4 head-groups (4 heads per core).
Each core computes a partial [S, D] output (its 4 heads pushed through
the O-projection); the host sums the 4 partials per batch and adds bo.

Layouts (per core, all matmul operands at partition base 0):
  qt  [128, pair, S]   q^T, two heads stacked on partitions (d dims)
  ktz [128, head, S]   k^T zero-padded: even heads live on partitions
                       0-63 (64-127 zero), odd heads on 64-127 - the
                       scores matmul is then a plain K=128 matmul
                       against the pair-stacked qt.
  v2  [128, head, kj, 65]  V blocks with an appended ones column
                       (produces the softmax denominator for free).
  scores^T [sk, sq] in PSUM -> exp on ScalarE -> bf16 tiles ->
  attnU^T [65, sq] accumulated with V2 stationary (N=512 moving);
  rowsum = row 64.  1/rowsum = exp(-ln(rowsum)) on ScalarE (one shared
  Exp+Ln table set, patched below), written in bf16 into a pre-zeroed
  SBUF tile; a plain K=128 ones-stationary matmul then broadcasts it
  across partitions (no HBM round-trip, no gpsimd ucode).

Scheduling: the PE instruction queue is FIFO in emission order, so the
kj loop is software-pipelined depth-2 (scores(i) is emitted before
attnU(i-2), hiding the ScalarE exp + gpsimd affine_select latency) and
independent projection / output-projection / finalize units are
interleaved as fillers so the PE never waits on the exp chain. Input
DMAs are merged into few large transfers, staged in waves via WAR
dependencies (one-column overlap with the previous wave's readers) so
the SDMA packet round-robin cannot starve the first wave. Dummy
matmuls keep the PE's HAM activity window hot through the DMA-bound
prologue and the finalize tail.
"""

import numpy as np
import ml_dtypes

import concourse.bass as bass
import concourse.bacc as bacc
import concourse.tile as tile
import concourse.mybir as mybir
from concourse import library_config
from concourse.bass_utils import run_bass_kernel_spmd

BF16 = mybir.dt.bfloat16
F32 = mybir.dt.float32
NPBF16 = ml_dtypes.bfloat16
AF = mybir.ActivationFunctionType

B = 2
S = 2048
D = 1024
H = 16
HD = 64
NCORES = 8
HPC = 4            # heads per core
NPAIRS = 2         # head pairs per core
NQ = S // 128      # 16 query/key blocks of 128
QCH = 512          # sq chunk width
NCH = S // QCH     # 4 chunks
KT = D // 128      # 8 contraction tiles for projections
import os
NWARM = int(os.environ.get("NWARM", "72"))  # PE warmup matmuls
NTAILWARM = int(os.environ.get("NTAILWARM", "22"))
BCAST = os.environ.get("BCAST", "pemm")  # pemm | gpsimd | dma
ROWTILE = os.environ.get("ROWTILE", "0") == "1"
RECIP = os.environ.get("RECIP", "act")  # act | div | dve | dvenat


def _emit(tc: tile.TileContext, io: dict):
    nc = tc.nc

    persist = tc.alloc_tile_pool(name="persist", bufs=1)

    # ---- constants ----
    ones128 = persist.tile([128, 128], BF16, name="ones128")
    nc.gpsimd.memset(ones128, 1.0)
    ones64b = persist.tile([128, 64], BF16, name="ones64b")
    nc.gpsimd.memset(ones64b, 1.0)
    onesrow = persist.tile([128, 2, QCH], BF16, name="onesrow")
    nc.gpsimd.memset(onesrow[64:65, :, :], 1.0)

    # Prefetch the Exp activation table (~2.7us) while input DMAs stream.
    actwarm = persist.tile([1, 8], F32, name="actwarm")
    nc.vector.memset(actwarm, 0.0)
    actwarm2 = persist.tile([1, 8], F32, name="actwarm2")
    nc.scalar.activation(actwarm2[0:1, :], actwarm[0:1, :], AF.Exp)

    if BCAST == "gpsimd":
        # gpsimd library with partition_broadcast (used by finalize)
        nc.gpsimd.load_library(library_config.attn)

    # ---- persistent SBUF arrays ----
    qt = persist.tile([128, NPAIRS, S], BF16, name="qt")
    if ROWTILE:
        kt = persist.tile([128, NPAIRS, S], BF16, name="kt")
    else:
        ktz = persist.tile([128, HPC, S], BF16, name="ktz")
        for h in range(HPC):  # zero the unused half of each ktz head
            half = slice(64, 128) if h % 2 == 0 else slice(0, 64)
            nc.gpsimd.memset(ktz[half, h, :], 0.0)
    v2 = persist.tile([128, HPC, NQ, 65], BF16, name="v2")
    fs = persist.tile([128, HPC, NQ, 65], BF16, name="fs")
    att = persist.tile([128, NPAIRS, S], BF16, name="att")

    qts = persist.tile([128, KT, S], BF16, name="qts")
    kts = persist.tile([128, KT, S], BF16, name="kts")
    vts = persist.tile([128, KT, S], BF16, name="vts")
    wqt = persist.tile([128, KT, 256], BF16, name="wqt")
    wkt = persist.tile([128, KT, 256], BF16, name="wkt")
    wvt = persist.tile([128, KT, 256], BF16, name="wvt")
    wot = persist.tile([128, NPAIRS, D], BF16, name="wot")
    # q/k biases as per-partition columns (dims live on partitions in
    # the qt/ktz layout): bqp[:, p] is the [128, 1] bias for pair p,
    # added during the PSUM->SBUF cast via tensor_scalar - no rank-1
    # bias matmuls on the PE.
    bqp = persist.tile([128, NPAIRS], F32, name="bqp")
    bkp = persist.tile([128, NPAIRS], F32, name="bkp")

    # ---- input DMAs: few, large transfers; spread across engine queues
    # so no compute engine's FIFO gets blocked behind a slow trigger.
    # V-side first (vproj runs first), then QK wave 1, then wave 2.
    vt3 = io["VT"].rearrange("(t p) s -> p t s", t=KT)
    qt3 = io["QT"].rearrange("(t p) s -> p t s", t=KT)
    kt3 = io["KT"].rearrange("(t p) s -> p t s", t=KT)
    # vts wave 1 is the critical first data (vproj runs first): split it
    # across the sync and scalar HWDGE rings so it gets ~2x the
    # per-ring bandwidth share during the congested prologue.
    nc.sync.dma_start(wvt[:, :, :], io["WvT"].rearrange("(t p) n -> p t n", t=KT))
    nc.sync.dma_start(vts[:, 0:4, 0:QCH], vt3[:, 0:4, 0:QCH])
    nc.scalar.dma_start(vts[:, 4:8, 0:QCH], vt3[:, 4:8, 0:QCH])
    nc.gpsimd.dma_start(wqt[:, :, :], io["WqT"].rearrange("(t p) n -> p t n", t=KT))
    nc.gpsimd.dma_start(wkt[:, :, :], io["WkT"].rearrange("(t p) n -> p t n", t=KT))
    nc.scalar.dma_start(bqp, io["bqT"].rearrange("g p -> p g"))
    nc.scalar.dma_start(bkp, io["bkT"].rearrange("g p -> p g"))
    nc.scalar.dma_start(qts[:, :, 0:QCH], qt3[:, :, 0:QCH])
    nc.gpsimd.dma_start(kts[:, :, 0:QCH], kt3[:, :, 0:QCH])
    nc.scalar.dma_start(wot[:, :, :],
                        io["WoT"].rearrange("(g p) n -> p g n", g=NPAIRS))
    # Later waves are staged so they can't steal HBM bandwidth from the
    # critical wave-1 data (the SDMA engines round-robin over ALL queued
    # DMAs at packet granularity). Each wave's dst overlaps its
    # predecessor by ONE column: tiny Vector "dummy reads" of that
    # column complete the moment wave 1 lands, and the WAR dependency on
    # them releases the qk wave-2 triggers right then (~20us); the final
    # qk wave chains behind wave 2 via the same-column WAW. vts wave 2
    # stays anchored on vproj(0)'s reads.
    def dma_v_wave2():
        nc.sync.dma_start(vts[:, :, QCH - 1:], vt3[:, :, QCH - 1:])

    def dma_qk_wave(w):
        s = w * QCH - 1
        e = (w + 1) * QCH
        nc.sync.dma_start(qts[:, :, s:e], qt3[:, :, s:e])
        nc.gpsimd.dma_start(kts[:, :, s:e], kt3[:, :, s:e])

    nc.gpsimd.memset(v2[:, :, :, 64:65], 1.0)  # ones column
    nc.gpsimd.memset(fs[:, :, NQ - 1, :], 0.0)  # suffix chain seed

    pb_s = tc.alloc_tile_pool(name="pb_scores", bufs=2, space="PSUM")
    pb_a = tc.alloc_tile_pool(name="pb_attnu", bufs=2, space="PSUM")

    # PE warmup: the first ~7us are DMA-bound with the PE idle, so the
    # HAM clock gate holds the array at 1.2 GHz when real work starts.
    # Dummy matmuls on the resident ones128 keep the activity window hot
    # (outputs are never read).
    for i in range(NWARM):
        w = pb_s.tile([128, 2, QCH], F32, tag="sps", name=f"warm{i}")
        nc.tensor.matmul(w[:, 0, 0:128], ones128, ones128,
                         start=True, stop=True)
    pb_e = tc.alloc_tile_pool(name="pb_exp", bufs=7)
    pb_r = tc.alloc_tile_pool(name="pb_recip", bufs=2)
    rec_bufs = []
    if BCAST == "pemm":
        # rec buffers: only row 64 is ever written (the reciprocal); the
        # other 127 rows stay zero so a plain K=128 ones-stationary
        # matmul broadcasts row 64 across output partitions. Two
        # persistent buffers, alternated across finalize_pair calls.
        for i in range(2):
            z = persist.tile([128, 2, QCH], BF16, name=f"recbuf{i}")
            nc.gpsimd.memset(z, 0.0)
            rec_bufs.append(z)
    rec_idx = [0]

    def vproj_unit(st):
        """V projection for key block st -> v2 tiles (one strided cast)."""
        psv_t = pb_s.tile([128, 2, 4, 64], F32, tag="sps", name=f"ps_v{st}")
        ps_v = psv_t[:, 0, :, :]
        for t in range(KT):
            nc.tensor.matmul(ps_v,
                             vts[:, t, st * 128:(st + 1) * 128],
                             wvt[:, t, :], start=(t == 0),
                             stop=(t == KT - 1))
        nc.vector.tensor_copy(v2[:, :, st, 0:64], ps_v)

    def vproj(c):
        for st in range(4 * c, 4 * c + 4):
            vproj_unit(st)

    def qproj_unit(c, p):
        sq = slice(c * QCH, (c + 1) * QCH)
        psq_t = pb_s.tile([128, 2, QCH], F32, tag="sps", name=f"ps_q{p}_{c}")
        ps_q = psq_t[:, 0, :]
        for t in range(KT):
            nc.tensor.matmul(ps_q, wqt[:, t, p * 128:(p + 1) * 128],
                             qts[:, t, sq], start=(t == 0),
                             stop=(t == KT - 1))
        nc.vector.tensor_scalar_add(qt[:, p, sq], ps_q, bqp[:, p:p + 1])

    def kproj_unit(c, p):
        sq = slice(c * QCH, (c + 1) * QCH)
        psk_t = pb_s.tile([128, 2, QCH], F32, tag="sps", name=f"ps_k{p}_{c}")
        ps_k = psk_t[:, 0, :]
        for t in range(KT):
            nc.tensor.matmul(ps_k, wkt[:, t, p * 128:(p + 1) * 128],
                             kts[:, t, sq], start=(t == 0),
                             stop=(t == KT - 1))
        if ROWTILE:
            nc.vector.tensor_scalar_add(kt[:, p, sq], ps_k, bkp[:, p:p + 1])
        else:
            nc.vector.tensor_scalar_add(ktz[0:64, 2 * p, sq], ps_k[0:64, :],
                                        bkp[0:64, p:p + 1])
            nc.vector.tensor_scalar_add(ktz[64:128, 2 * p + 1, sq],
                                        ps_k[64:128, :],
                                        bkp[64:128, p:p + 1])

    def qkproj(c):
        for p in range(NPAIRS):
            qproj_unit(c, p)
            kproj_unit(c, p)

    def qk_fillers(c):
        return [lambda p=p, f=f: f(c, p)
                for p in range(NPAIRS) for f in (qproj_unit, kproj_unit)]

    def fs_segment(qlo, qhi, split=False):
        """fs[q] for q = qhi-1 .. qlo (needs v2[qlo+1..qhi] and fs[qhi]).
        vproj runs in reverse block order (3,2,1) so these segments
        build incrementally, each right after the v2 blocks it needs.
        The last (longest) segment splits heads across Vector/GpSimd."""
        for h in range(HPC):
            eng = nc.gpsimd if (split and h >= 2) else nc.vector
            for q in range(qhi - 1, qlo - 1, -1):
                eng.tensor_add(fs[:, h, q, :], fs[:, h, q + 1, :],
                               v2[:, h, q + 1, :])

    aups_tiles = {}

    def chunk_loop(c, fillers=(), pre_special=None, tailwarm=0,
                   early_fillers=()):
        """scores -> exp -> attnU^T for chunk c, both pairs.

        Software-pipelined: scores+exp for unit i+1 are emitted before
        attnU of unit i, so the PE streams the next scores while the
        ScalarE exp chain drains.  Filler units (independent PE work)
        are spread evenly through the loop to absorb the exp backlog.

        finalize work is emitted per-pair: pair 0's finalize lands a few
        units into pair 1's run (hiding its Ln/Exp latency behind live
        PE work); pair 1's finalize is handed to the NEXT chunk loop via
        pre_special (or emitted at the end for the last chunk).
        """
        fillers = list(fillers)
        nkj = 4 * c + 4
        units = [(p, kj) for p in range(NPAIRS) for kj in range(nkj)]
        nu = len(units)
        fill_at = {}
        for j, f in enumerate(early_fillers):  # fixed positions 0,1,2..
            fill_at.setdefault(min(nu - 1, j), []).append(f)
        if pre_special is not None:  # e.g. previous chunk's p1 finalize
            fill_at.setdefault(min(nu - 1, 2), []).append(pre_special)
        if c > 0:  # this chunk's p0 finalize, 3 units into p1's run
            fill_at.setdefault(min(nu - 1, nkj + 2), []).append(
                lambda: finalize_pair(c, 0))
        # regular fillers spread from position 3 onward
        if fillers:
            for j, f in enumerate(fillers):
                fill_at.setdefault(
                    min(nu - 1, 3 + (j * (nu - 3)) // len(fillers)), []).append(f)

        exts = {}

        def emit_scores_exp(p, kj):
            c0 = max(kj - 4 * c, 0) * 128   # first valid col in chunk
            sps = pb_s.tile([128, 2, QCH], F32, tag="sps",
                            name=f"sps{p}_{c}_{kj}")
            for hl in range(2):
                h0 = hl * 64
                if ROWTILE:
                    # K=64 row-tiled: the two heads run concurrently in
                    # the PE array (rows 0-63 / 64-127).
                    nc.tensor.matmul(
                        sps[:, hl, c0:QCH],
                        kt[h0:h0 + 64, p, kj * 128:(kj + 1) * 128],
                        qt[h0:h0 + 64, p, c * QCH + c0:(c + 1) * QCH],
                        start=True, stop=True)
                else:
                    nc.tensor.matmul(
                        sps[:, hl, c0:QCH],
                        ktz[:, 2 * p + hl, kj * 128:(kj + 1) * 128],
                        qt[:, p, c * QCH + c0:(c + 1) * QCH],
                        start=True, stop=True)
            ext = pb_e.tile([128, 2, QCH], BF16, tag="ext",
                            name=f"ext{p}_{c}_{kj}")
            nc.scalar.activation(ext[:, :, c0:QCH], sps[:, :, c0:QCH],
                                 AF.Exp, scale=0.125)
            if kj >= 4 * c:  # diagonal block: masked exp entries -> 1.0
                # one affine_select covers both heads: the hl dim gets
                # a zero stride in the affine pattern
                nc.gpsimd.affine_select(
                    out=ext[:, :, c0:c0 + 128],
                    in_=ext[:, :, c0:c0 + 128],
                    compare_op=mybir.AluOpType.is_ge,
                    fill=1.0, base=0,
                    pattern=[[0, 2], [1, 128]], channel_multiplier=-1)
            exts[(p, kj)] = (ext, c0)

        def emit_attnu(p, kj):
            ext, c0 = exts.pop((p, kj))
            if kj == 0:
                # allocate here (not at scores emission) so a chunk's
                # first scores/exp aren't gated on the previous chunk's
                # finalize muls releasing this pool slot
                aups_tiles[(p, c)] = pb_a.tile(
                    [128, 2, QCH], F32, tag="aups", name=f"aups{p}_{c}")
            aups = aups_tiles[(p, c)]
            for hl in range(2):
                # masked cols < c0 get their (block-constant)
                # contribution from the early FS matmuls below
                nc.tensor.matmul(
                    aups[0:65, hl, c0:QCH],
                    v2[:, 2 * p + hl, kj, :],
                    ext[:, hl, c0:QCH],
                    start=(kj == 0),
                    stop=(kj == nkj - 1 and c > 0))
            if kj == 0 and c > 0:
                # suffix adds commute with the accumulation: emit them
                # up front so finalize()'s recip can start the moment
                # the last attnU matmul lands
                for hl in range(2):
                    h = 2 * p + hl
                    for ql in range(4):
                        qi = 4 * c + ql
                        if qi < NQ - 1:
                            nc.tensor.matmul(
                                aups[0:65, hl, ql * 128:(ql + 1) * 128],
                                fs[:, h, qi, :], ones128,
                                start=False, stop=False)

        # depth-2 software pipeline: attnU(i-2) is emitted after
        # scores(i), covering the ScalarE exp AND the gpsimd
        # affine_select latency of the diagonal blocks.
        for i, (p, kj) in enumerate(units):
            emit_scores_exp(p, kj)
            if i > 1:
                emit_attnu(*units[i - 2])
            for f in fill_at.get(i, ()):
                f()
        emit_attnu(*units[-2])
        emit_attnu(*units[-1])
        for i in range(tailwarm):
            w = pb_s.tile([128, 2, QCH], F32, tag="sps", name=f"twarm{c}_{i}")
            nc.tensor.matmul(w[:, 0, 0:128], ones128, ones128,
                             start=True, stop=True)

    def fs_close(c, p):
        """fs suffix matmuls closing the aups accumulation for chunk 0
        (fs is not yet computed when chunk 0's attnU is emitted)."""
        aups = aups_tiles[(p, c)]
        for hl in range(2):
            for ql in range(4):
                nc.tensor.matmul(
                    aups[0:65, hl, ql * 128:(ql + 1) * 128],
                    fs[:, 2 * p + hl, 4 * c + ql, :], ones128,
                    start=False, stop=(ql == 3))

    def finalize_pair(c, p, blockwise=False, fs_done=False):
        """rowsum reciprocal (ScalarE Ln/Exp) -> partition broadcast via
        a K=1 PE matmul into the UNUSED partitions 64-127 of the aups
        PSUM tile -> normalize into att for chunk c, pair p."""
        ch = slice(c * QCH, (c + 1) * QCH)
        aups = aups_tiles[(p, c)]
        if c == 0 and not fs_done:
            for hl in range(2):
                for ql in range(4):
                    nc.tensor.matmul(
                        aups[0:65, hl, ql * 128:(ql + 1) * 128],
                        fs[:, 2 * p + hl, 4 * c + ql, :], ones128,
                        start=False, stop=(ql == 3))
        if BCAST == "pemm":
            rec = rec_bufs[rec_idx[0] % 2]
            rec_idx[0] += 1
        else:
            rec = pb_r.tile([128, 2, QCH], F32, tag="lnr", name=f"rec{p}_{c}")
        if RECIP == "div":
            # single DVE divide: keeps the reciprocal chain off the
            # ScalarE queue, which is saturated with softmax exps
            nc.vector.tensor_tensor(rec[64:65, :, :], onesrow[64:65, :, :],
                                    aups[64:65, :, :],
                                    mybir.AluOpType.divide)
        else:
            lnr = pb_r.tile([128, 2, QCH], F32, tag="lnr", name=f"lnr{p}_{c}")
            nc.scalar.activation(lnr[64:65, :, :], aups[64:65, :, :], AF.Ln)
            nc.scalar.activation(rec[64:65, :, :], lnr[64:65, :, :],
                                 AF.Exp, scale=-1.0)
        if BCAST == "pemm":
            # broadcast 1/rowsum to all partitions: rows != 64 of rec
            # are zero, so ones128^T @ rec replicates row 64. Standard
            # (0,0) matmul into a borrowed scores-pool PSUM tile, then a
            # DVE copy to SBUF for the (PSUM x SBUF) normalize muls.
            rep_ps = pb_s.tile([128, 2, QCH], F32, tag="sps",
                               name=f"repps{p}_{c}")
            for hl in range(2):
                nc.tensor.matmul(rep_ps[:, hl, :], ones128, rec[:, hl, :],
                                 start=True, stop=True)
            rep = pb_r.tile([128, 2, QCH], BF16, tag="rep", name=f"rep{p}_{c}")
            nc.vector.tensor_copy(rep[0:64, :, :], rep_ps[0:64, :, :])
            rl = 0
        else:
            rep = pb_r.tile([128, 2, QCH], F32, tag="rep", name=f"rep{p}_{c}")
            r = p * NCH + c
            nc.sync.dma_start(io["dscratch"][r:r + 1, :], rec[64:65, :, :])
            nc.sync.dma_start(
                rep[0:64, :, :],
                io["dscratch"][r:r + 1, :].rearrange(
                    "r (h q) -> r h q", h=2).broadcast_to([64, 2, QCH]))
            rl = 0
        if blockwise:
            # per-128-query-block muls so the tail outproj units can
            # start as soon as their block is normalized
            for ql in range(4):
                cqs = slice(c * QCH + ql * 128, c * QCH + (ql + 1) * 128)
                for hl in range(2):
                    nc.vector.tensor_mul(
                        att[hl * 64:(hl + 1) * 64, p, cqs],
                        aups[0:64, hl, ql * 128:(ql + 1) * 128],
                        rep[rl:rl + 64, hl, ql * 128:(ql + 1) * 128])
        else:
            for hl in range(2):
                nc.vector.tensor_mul(
                    att[hl * 64:(hl + 1) * 64, p, ch],
                    aups[0:64, hl, :],
                    rep[rl:rl + 64, hl, :])

    def finalize(c, blockwise=False):
        for p in range(NPAIRS):
            finalize_pair(c, p, blockwise)

    ob_tiles = {}

    def outproj_unit(st, dc):
        pso = pb_s.tile([128, 2, QCH], F32, tag="sps", name=f"pso{st}_{dc}")
        for p in range(NPAIRS):
            # K=128 contraction = both heads of the pair stacked
            nc.tensor.matmul(
                pso[:, 0, :],
                att[:, p, st * 128:(st + 1) * 128],
                wot[:, p, dc * 512:(dc + 1) * 512],
                start=(p == 0), stop=(p == NPAIRS - 1))
        if st not in ob_tiles:
            ob_tiles[st] = pb_e.tile([128, 2, QCH], BF16, tag="ob",
                                     name=f"ob{st}")
        ob = ob_tiles.pop(st) if dc == 1 else ob_tiles[st]
        if st >= 12:  # tail blocks: ScalarE is idle after the last exp,
            # so do the PSUM->SBUF casts there and keep VectorE free for
            # the final normalize muls
            nc.scalar.copy(ob[:, dc, :], pso[:, 0, :])
        else:
            nc.vector.tensor_copy(ob[:, dc, :], pso[:, 0, :])
        if dc == 1:  # one output DMA per 128-row block
            nc.gpsimd.dma_start(io["out"][st * 128:(st + 1) * 128, :], ob)

    def outproj(c):
        for st in range(4 * c, 4 * c + 4):
            for dc in range(2):
                outproj_unit(st, dc)

    def op_fillers(c):
        return [lambda st=st, dc=dc: outproj_unit(st, dc)
                for st in range(4 * c, 4 * c + 4) for dc in range(2)]

    # Main schedule.  finalize(c) needs ALL v2 blocks (its FS suffix
    # sums reach to kj=15 - a true data dependency of the reference), so
    # finalizes wait until all v-projections are emitted; aups
    # double-buffering then bounds how many chunk loops may run before
    # the first finalize.  qkproj(c+1) and outproj(c-1) units are
    # interleaved into chunk_loop(c) as PE fillers.
    vproj(0)
    dma_v_wave2()
    qkproj(0)
    dma_qk_wave(1)
    chunk_loop(0)
    vproj(3)
    fs_segment(11, NQ - 1)
    vproj(2)
    fs_segment(7, 11)
    vproj(1)
    fs_segment(0, 7, split=True)
    qkproj(1)
    dma_qk_wave(2)
    # emit BOTH pairs' fs-closing matmuls before any recip chain, so
    # pair 1's PE work fills the Ln/Exp latency of pair 0's reciprocal
    fs_close(0, 0)
    fs_close(0, 1)
    for i in range(10):  # keep the PE's HAM window hot through the
        # first reciprocal chain (bc matmuls wait on ScalarE here)
        w = pb_s.tile([128, 2, QCH], F32, tag="sps", name=f"f0warm{i}")
        nc.tensor.matmul(w[:, 0, 0:128], ones128, ones128,
                         start=True, stop=True)
    finalize_pair(0, 0, fs_done=True)
    finalize_pair(0, 1, fs_done=True)
    # filler balance: chunks 1/2 are close to PE-bound already (each
    # exp-paced unit leaves only ~150ns of PE slack), while chunk 3 has
    # 32 exp-paced units - park the output-projection work there.
    chunk_loop(1, fillers=qk_fillers(2) + [lambda: dma_qk_wave(3)]
               + op_fillers(0))
    chunk_loop(2, fillers=qk_fillers(3),
               pre_special=lambda: finalize_pair(1, 1))
    chunk_loop(3, fillers=op_fillers(1) + op_fillers(2),
               pre_special=lambda: finalize_pair(2, 1), tailwarm=NTAILWARM)
    finalize_pair(3, 1, blockwise=True)
    outproj(3)

    pb_r.release()
    pb_e.release()
    pb_a.release()
    pb_s.release()
    persist.release()


_CACHED = None


def _patch_act_tables():
    """Make Exp and Ln resolve to the single combined table set so the
    per-chunk recip (Ln/Exp) doesn't thrash ACT_TABLE_LOADs against the
    softmax Exp calls. Set positions (= act_func_set_id) are preserved;
    only membership of Exp/Ln in other sets is hidden from the selector."""
    from concourse import hw_specs
    orig = hw_specs.get_activation_tables

    def patched(arch):
        t = dict(orig(arch))
        if "natural_log_exp_and_others" in t:
            for name in t:
                if name != "natural_log_exp_and_others":
                    t[name] = t[name] - {AF.Exp, AF.Ln}
        return t

    bacc.get_activation_tables = patched


def _build():
    global _CACHED
    if _CACHED is not None:
        return _CACHED
    _patch_act_tables()
    nc = bacc.Bacc("TRN2", target_bir_lowering=False, debug=False)
    io = {
        "QT": nc.dram_tensor("QT", [D, S], BF16, kind="ExternalInput").ap(),
        "KT": nc.dram_tensor("KT", [D, S], BF16, kind="ExternalInput").ap(),
        "VT": nc.dram_tensor("VT", [D, S], BF16, kind="ExternalInput").ap(),
        "WqT": nc.dram_tensor("WqT", [D, 256], BF16, kind="ExternalInput").ap(),
        "WkT": nc.dram_tensor("WkT", [D, 256], BF16, kind="ExternalInput").ap(),
        "WvT": nc.dram_tensor("WvT", [D, 256], BF16, kind="ExternalInput").ap(),
        "WoT": nc.dram_tensor("WoT", [256, D], BF16, kind="ExternalInput").ap(),
        "bqT": nc.dram_tensor("bqT", [NPAIRS, 128], F32,
                              kind="ExternalInput").ap(),
        "bkT": nc.dram_tensor("bkT", [NPAIRS, 128], F32,
                              kind="ExternalInput").ap(),
        "out": nc.dram_tensor("out", [S, D], BF16, kind="ExternalOutput").ap(),
        "dscratch": nc.dram_tensor("dscratch", [NPAIRS * NCH, 2 * QCH], F32,
                                   kind="Internal").ap(),
    }
    with tile.TileContext(nc) as tc:
        _emit(tc, io)
    nc.compile()
    _CACHED = (nc, io)
    return _CACHED


def make_in_maps(Q, K, V, Wq, bq, Wk, bk, Wv, bv, Wo):
    """Build the 8 per-core input dicts (host-side sharding)."""
    Q = np.asarray(Q, np.float32)
    K = np.asarray(K, np.float32)
    V = np.asarray(V, np.float32)
    qt = [np.ascontiguousarray(Q[b].T).astype(NPBF16) for b in range(B)]
    kt = [np.ascontiguousarray(K[b].T).astype(NPBF16) for b in range(B)]
    vt = [np.ascontiguousarray(V[b].T).astype(NPBF16) for b in range(B)]
    in_maps = []
    for core in range(NCORES):
        b, g = divmod(core, 4)
        rows = slice(g * 256, (g + 1) * 256)
        in_maps.append({
            "QT": qt[b], "KT": kt[b], "VT": vt[b],
            "WqT": np.ascontiguousarray(np.asarray(Wq, np.float32)[rows].T).astype(NPBF16),
            "WkT": np.ascontiguousarray(np.asarray(Wk, np.float32)[rows].T).astype(NPBF16),
            "WvT": np.ascontiguousarray(np.asarray(Wv, np.float32)[rows].T).astype(NPBF16),
            "WoT": np.ascontiguousarray(np.asarray(Wo, np.float32)[:, rows].T).astype(NPBF16),
            "bqT": np.ascontiguousarray(
                np.asarray(bq, np.float32)[rows].reshape(NPAIRS, 128)),
            "bkT": np.ascontiguousarray(
                np.asarray(bk, np.float32)[rows].reshape(NPAIRS, 128)),
        })
    return in_maps


def kernel(Q, K, V, mask, Wq, bq, Wk, bk, Wv, bv, Wo, bo, _results_hook=None):
    nc, _io = _build()
    in_maps = make_in_maps(Q, K, V, Wq, bq, Wk, bk, Wv, bv, Wo)
    res = run_bass_kernel_spmd(nc, in_maps, core_ids=list(range(NCORES)))
    if _results_hook is not None:
        _results_hook(res)
    out = np.zeros((B, S, D), np.float32)
    for core in range(NCORES):
        out[core // 4] += np.asarray(res.results[core]["out"], np.float32)
    # bv passes straight through the softmax average; fold it (and bo)
    # into the output bias here.
    out += np.asarray(bo, np.float32) + \
        np.asarray(bv, np.float32) @ np.asarray(Wo, np.float32).T
    return out
